# revision 7
# baseline (speedup 1.0000x reference)
"""FastSelfAttention Trainium2 kernel.

Reference computation (B=4, S=4096, D=1024):
    h  = layer_norm(hidden_states, g, b)
    q  = h @ Wq.T ; k = h @ Wk.T ; v = q
    qw = exp((q @ wq_att) / sqrt(D) + mask)
    pq = cumsum(qw * q, S) / cumsum(qw, S)
    mk = pq * k
    kw = exp((mk @ wk_att) / sqrt(D) + mask)
    pk = cumsum(kw * mk, S) / cumsum(kw, S)
    out = pk * v

Sharding: 8 cores = 4 batches x 2 halves of the feature (e) dimension.
Each core owns its batch's full sequence and half of the q/k output
features. Layout on device is feature-major [e, s]; cumsum runs along
the free (s) axis via the DVE tensor_tensor_scan primitive, chained
across s-chunks with carry columns.

LayerNorm is folded into the projections:
    q[e,s] = rstd[s] * (sum_d W'q[e,d] (h[d,s]-mu[s])) + cq[e]
with W'q = Wq*g, cq = Wq@b precomputed on host; the -mu term is a
rank-1 matmul accumulated into the same PSUM tile.

The second pooling's logit l2[s] = sum_e wk[e]*mk[e,s] needs the full
e range: each core computes its half and a pairwise AllReduce
([[0,1],[2,3],[4,5],[6,7]]) combines them.
"""

import numpy as np

import concourse.bass as bass
import concourse.bacc as bacc
import concourse.mybir as mybir
import concourse.tile as tile
from concourse.bass_utils import run_bass_kernel_spmd

dt = mybir.dt
AF = mybir.ActivationFunctionType
OP = mybir.AluOpType

B, S, D = 4, 4096, 1024
EH = D // 2          # e-half per core
NC = 8               # cores
SC = 512             # s-chunk
NSC = S // SC        # 8 s-chunks
ND = D // 128        # 8 d-chunks
NE = EH // 128       # 4 e-chunks per core
INV_SQRT_D = 1.0 / np.sqrt(np.float32(D))
EPS = 1e-5

_prog_cache = {}


def _build_program():
    if "nc" in _prog_cache:
        return _prog_cache["nc"]

    nc = bacc.Bacc("TRN2", num_devices=NC)
    f32, f32r, bf16 = dt.float32, dt.float32r, dt.bfloat16

    # ---- external I/O ----
    hT = nc.dram_tensor("hT", [D, S], f32, kind="ExternalInput")
    wqT = nc.dram_tensor("wqT", [D, EH], f32, kind="ExternalInput")
    wkT = nc.dram_tensor("wkT", [D, EH], f32, kind="ExternalInput")
    # per-partition constants, host layout [n, 128] -> SBUF [128, n]
    cq_in = nc.dram_tensor("cq", [NE, 128], f32, kind="ExternalInput")
    ck_in = nc.dram_tensor("ck", [NE, 128], f32, kind="ExternalInput")
    vqp_in = nc.dram_tensor("vqp", [ND, 128], f32, kind="ExternalInput")
    wkp_in = nc.dram_tensor("wkp", [NE, 128], f32, kind="ExternalInput")
    # rows
    colsq_in = nc.dram_tensor("colsq", [1, EH], f32, kind="ExternalInput")
    colsk_in = nc.dram_tensor("colsk", [1, EH], f32, kind="ExternalInput")
    colsvq_in = nc.dram_tensor("colsvq", [1, 1], f32, kind="ExternalInput")
    mrow1_in = nc.dram_tensor("mrow1", [1, S], f32, kind="ExternalInput")
    mrow2_in = nc.dram_tensor("mrow2", [1, S], f32, kind="ExternalInput")
    ones_in = nc.dram_tensor("ones", [1, 128], f32, kind="ExternalInput")

    outT = nc.dram_tensor("outT", [EH, S], f32, kind="ExternalOutput")

    with tile.TileContext(nc) as tc:
        with (
            tc.tile_pool(name="const", bufs=1) as cpool,
            tc.tile_pool(name="persist", bufs=1) as ppool,
            tc.tile_pool(name="rows", bufs=1) as rows,
            tc.tile_pool(name="bc", bufs=2) as bc,
            tc.tile_pool(name="psA", bufs=2, space="PSUM") as psA,
            tc.tile_pool(name="psB", bufs=2, space="PSUM") as psB,
            tc.tile_pool(name="psR", bufs=2, space="PSUM") as psR,
            tc.tile_pool(name="psL2", bufs=1, space="PSUM") as psL2,
            tc.tile_pool(name="dram", bufs=1, space="DRAM") as dpool,
        ):
            # ---- constants (resident) ----
            cq_t = cpool.tile([128, NE], f32, tag="cq")
            ck_t = cpool.tile([128, NE], f32, tag="ck")
            vqp_t = cpool.tile([128, ND], f32r, tag="vqp")
            wkp_t = cpool.tile([128, NE], f32r, tag="wkp")
            nc.gpsimd.dma_start(out=cq_t[:], in_=cq_in.transpose([1, 0]))
            nc.gpsimd.dma_start(out=ck_t[:], in_=ck_in.transpose([1, 0]))
            nc.gpsimd.dma_start(out=vqp_t[:], in_=vqp_in.transpose([1, 0]).bitcast(f32r))
            nc.gpsimd.dma_start(out=wkp_t[:], in_=wkp_in.transpose([1, 0]).bitcast(f32r))

            colsq_t = cpool.tile([1, EH], f32r, tag="colsq")
            colsk_t = cpool.tile([1, EH], f32r, tag="colsk")
            colsvq_t = cpool.tile([1, 1], f32r, tag="colsvq")
            nc.gpsimd.dma_start(out=colsq_t[:], in_=colsq_in[:].bitcast(f32r))
            nc.gpsimd.dma_start(out=colsk_t[:], in_=colsk_in[:].bitcast(f32r))
            nc.gpsimd.dma_start(out=colsvq_t[:], in_=colsvq_in[:].bitcast(f32r))

            ones_k1 = cpool.tile([1, 128], f32r, tag="ones_k1")
            nc.gpsimd.dma_start(out=ones_k1[:], in_=ones_in[:].bitcast(f32r))
            ones_d = cpool.tile([128, 1], f32r, tag="ones_d")
            nc.gpsimd.dma_start(out=ones_d[:], in_=ones_in.transpose([1, 0]).bitcast(f32r))
            eps_t = cpool.tile([1, 1], f32, tag="eps")
            nc.vector.memset(eps_t[:], EPS)

            # ---- persistent state ----
            carry_q = ppool.tile([128, NE], f32, tag="carry_q")
            carry_k = ppool.tile([128, NE], f32, tag="carry_k")
            carry_d = ppool.tile([1, 2], f32, tag="carry_d")
            nc.vector.memset(carry_q[:], 0.0)
            nc.vector.memset(carry_k[:], 0.0)
            nc.vector.memset(carry_d[:], 0.0)

            l2p_dram = dpool.tile([1, S], f32, tag="l2p")
            l2f_dram = dpool.tile([1, S], f32, tag="l2f")
            q_dram = dpool.tile([EH, S], bf16, tag="q_dram")
            mk_dram = dpool.tile([EH, S], bf16, tag="mk_dram")

            # ================= sweep 1 =================
            with (
                tc.tile_pool(name="wpool", bufs=1) as wpool,
                tc.tile_pool(name="ht", bufs=2) as htpool,
                tc.tile_pool(name="wk1", bufs=2) as wk1,
            ):
                wq_t = wpool.tile([128, ND, EH], f32r, tag="wq")
                wk_t = wpool.tile([128, ND, EH], f32r, tag="wk")
                for d in range(ND):
                    nc.gpsimd.dma_start(
                        out=wq_t[:, d, :],
                        in_=wqT[d * 128:(d + 1) * 128, :].bitcast(f32r))
                    nc.gpsimd.dma_start(
                        out=wk_t[:, d, :],
                        in_=wkT[d * 128:(d + 1) * 128, :].bitcast(f32r))

                for c in range(NSC):
                    s0 = c * SC
                    ht_t = htpool.tile([128, ND, SC], f32r, tag="ht")
                    for d in range(ND):
                        nc.sync.dma_start(
                            out=ht_t[:, d, :],
                            in_=hT[d * 128:(d + 1) * 128, s0:s0 + SC].bitcast(f32r))

                    # ---- stats ----
                    sx_ps = psR.tile([1, SC], f32, tag="srow")
                    for d in range(ND):
                        nc.tensor.matmul(sx_ps[:], ones_d[:], ht_t[:, d, :],
                                         start=(d == 0), stop=(d == ND - 1))
                    sxx_ps = psR.tile([1, SC], f32, tag="srow")
                    for d in range(ND):
                        sq_t = wk1.tile([128, SC], f32r, tag="u1")
                        nc.scalar.activation(
                            sq_t[:], ht_t[:, d, :].bitcast(f32), AF.Square)
                        nc.tensor.matmul(sxx_ps[:], ones_d[:], sq_t[:],
                                         start=(d == 0), stop=(d == ND - 1))

                    negmu = rows.tile([1, SC], f32r, tag="negmu")
                    nc.vector.tensor_scalar_mul(negmu[:], sx_ps[:], -1.0 / D)
                    musq = rows.tile([1, SC], f32, tag="musq")
                    nc.scalar.activation(musq[:], sx_ps[:], AF.Square, scale=1.0 / D)
                    var = rows.tile([1, SC], f32, tag="var")
                    nc.vector.scalar_tensor_tensor(
                        var[:], sxx_ps[:], 1.0 / D, musq[:], OP.mult, OP.subtract)
                    sd = rows.tile([1, SC], f32, tag="sd")
                    nc.scalar.activation(sd[:], var[:], AF.Sqrt, bias=eps_t[:])
                    rstd = rows.tile([1, SC], f32, tag="rstd")
                    rscr = rows.tile([1, SC], f32, tag="rscr")
                    nc.vector.reciprocal_approx_accurate(rstd[:], sd[:], rscr[:])
                    rstd_r = rows.tile([1, SC], f32r, tag="rstd_r")
                    nc.vector.tensor_copy(rstd_r[:], rstd[:])

                    rb_ps = psB.tile([128, SC], f32, tag="bcast")
                    nc.tensor.matmul(rb_ps[:], ones_k1[:], rstd_r[:],
                                     start=True, stop=True)
                    rstd_b = bc.tile([128, SC], f32, tag="rstd_b")
                    nc.scalar.copy(rstd_b[:], rb_ps[:])

                    # ---- l1 row (query attention logit) ----
                    l1_ps = psR.tile([1, SC], f32, tag="srow")
                    for d in range(ND):
                        nc.tensor.matmul(l1_ps[:], vqp_t[:, d:d + 1], ht_t[:, d, :],
                                         start=(d == 0), stop=False)
                    nc.tensor.matmul(l1_ps[:], colsvq_t[:], negmu[:],
                                     start=False, stop=True)
                    l1a = rows.tile([1, SC], f32, tag="l1a")
                    nc.vector.tensor_mul(l1a[:], l1_ps[:], rstd[:])
                    m1s = rows.tile([1, SC], f32, tag="m1s")
                    nc.sync.dma_start(out=m1s[:], in_=mrow1_in[:, s0:s0 + SC])
                    l1b = rows.tile([1, SC], f32, tag="l1b")
                    nc.vector.tensor_add(l1b[:], l1a[:], m1s[:])
                    qw = rows.tile([1, SC], f32, tag="qw")
                    nc.scalar.activation(qw[:], l1b[:], AF.Exp)
                    qw_r = rows.tile([1, SC], f32r, tag="qw_r")
                    nc.vector.tensor_copy(qw_r[:], qw[:])

                    qb_ps = psB.tile([128, SC], f32, tag="bcast")
                    nc.tensor.matmul(qb_ps[:], ones_k1[:], qw_r[:],
                                     start=True, stop=True)
                    qw_b = bc.tile([128, SC], f32, tag="qw_b")
                    nc.scalar.copy(qw_b[:], qb_ps[:])

                    # den1 scan + reciprocal + broadcast
                    den1 = rows.tile([1, SC], f32, tag="den1")
                    init1 = 0.0 if c == 0 else carry_d[:, 0:1]
                    nc.vector.tensor_tensor_scan(
                        den1[:], qw[:], qw[:], init1, OP.add, OP.bypass)
                    nc.vector.tensor_copy(carry_d[:, 0:1], den1[:, SC - 1:SC])
                    rden1 = rows.tile([1, SC], f32, tag="rden1")
                    nc.vector.reciprocal_approx_accurate(rden1[:], den1[:], rscr[:])
                    rden1_r = rows.tile([1, SC], f32r, tag="rden1_r")
                    nc.vector.tensor_copy(rden1_r[:], rden1[:])
                    db_ps = psB.tile([128, SC], f32, tag="bcast")
                    nc.tensor.matmul(db_ps[:], ones_k1[:], rden1_r[:],
                                     start=True, stop=True)
                    rden1_b = bc.tile([128, SC], f32, tag="rden1_b")
                    nc.scalar.copy(rden1_b[:], db_ps[:])

                    # ---- per e-chunk: projections, pool1, mk, l2 partial ----
                    l2_ps = psL2.tile([1, SC], f32, tag="l2")
                    for e in range(NE):
                        qmm_ps = psA.tile([128, SC], f32, tag="proj")
                        for d in range(ND):
                            nc.tensor.matmul(
                                qmm_ps[:], wq_t[:, d, e * 128:(e + 1) * 128],
                                ht_t[:, d, :], start=(d == 0), stop=False)
                        nc.tensor.matmul(
                            qmm_ps[:], colsq_t[:, e * 128:(e + 1) * 128], negmu[:],
                            start=False, stop=True)
                        qr_t = wk1.tile([128, SC], f32, tag="q")
                        nc.vector.tensor_mul(qr_t[:], qmm_ps[:], rstd_b[:])
                        q_t = wk1.tile([128, SC], f32, tag="q")
                        nc.scalar.activation(q_t[:], qr_t[:], AF.Identity,
                                             bias=cq_t[:, e:e + 1])
                        qb_t = wk1.tile([128, SC], bf16, tag="qb")
                        nc.scalar.activation(qb_t[:], q_t[:], AF.Copy)
                        nc.sync.dma_start(
                            out=q_dram[e * 128:(e + 1) * 128, s0:s0 + SC], in_=qb_t[:])

                        kmm_ps = psA.tile([128, SC], f32, tag="proj")
                        for d in range(ND):
                            nc.tensor.matmul(
                                kmm_ps[:], wk_t[:, d, e * 128:(e + 1) * 128],
                                ht_t[:, d, :], start=(d == 0), stop=False)
                        nc.tensor.matmul(
                            kmm_ps[:], colsk_t[:, e * 128:(e + 1) * 128], negmu[:],
                            start=False, stop=True)
                        kr_t = wk1.tile([128, SC], f32, tag="k")
                        nc.vector.tensor_mul(kr_t[:], kmm_ps[:], rstd_b[:])
                        k_t = wk1.tile([128, SC], f32, tag="k")
                        nc.scalar.activation(k_t[:], kr_t[:], AF.Identity,
                                             bias=ck_t[:, e:e + 1])

                        u1_t = wk1.tile([128, SC], f32, tag="u1")
                        nc.vector.tensor_mul(u1_t[:], qw_b[:], q_t[:])
                        n1_t = wk1.tile([128, SC], f32, tag="n1")
                        initq = 0.0 if c == 0 else carry_q[:, e:e + 1]
                        nc.vector.tensor_tensor_scan(
                            n1_t[:], u1_t[:], u1_t[:], initq, OP.add, OP.bypass)
                        nc.vector.tensor_copy(carry_q[:, e:e + 1], n1_t[:, SC - 1:SC])

                        pq_t = wk1.tile([128, SC], f32, tag="pq")
                        nc.gpsimd.tensor_mul(pq_t[:], n1_t[:], rden1_b[:])
                        mk_t = wk1.tile([128, SC], f32r, tag="mk")
                        nc.gpsimd.tensor_mul(mk_t[:], pq_t[:], k_t[:])
                        mkb_t = wk1.tile([128, SC], bf16, tag="mkb")
                        nc.scalar.activation(mkb_t[:], mk_t[:].bitcast(f32), AF.Copy)
                        nc.sync.dma_start(
                            out=mk_dram[e * 128:(e + 1) * 128, s0:s0 + SC], in_=mkb_t[:])
                        nc.tensor.matmul(l2_ps[:], wkp_t[:, e:e + 1], mk_t[:],
                                         start=(e == 0), stop=(e == NE - 1))

                    l2p_row = rows.tile([1, SC], f32, tag="l2p")
                    nc.vector.tensor_copy(l2p_row[:], l2_ps[:])
                    nc.sync.dma_start(out=l2p_dram[:, s0:s0 + SC], in_=l2p_row[:])

            # ================= allreduce =================
            nc.gpsimd.collective_compute(
                "AllReduce", OP.add,
                replica_groups=[[0, 1], [2, 3], [4, 5], [6, 7]],
                ins=[l2p_dram[:]], outs=[l2f_dram[:]],
            )

            # ================= sweep 2 =================
            with tc.tile_pool(name="wk2", bufs=2) as wk2:
                for c in range(NSC):
                    s0 = c * SC
                    l2s = rows.tile([1, SC], f32, tag="l2s")
                    nc.sync.dma_start(out=l2s[:], in_=l2f_dram[:, s0:s0 + SC])
                    m2s = rows.tile([1, SC], f32, tag="m2s")
                    nc.sync.dma_start(out=m2s[:], in_=mrow2_in[:, s0:s0 + SC])
                    lg2 = rows.tile([1, SC], f32, tag="lg2")
                    nc.vector.tensor_add(lg2[:], l2s[:], m2s[:])
                    kw = rows.tile([1, SC], f32, tag="kw")
                    nc.scalar.activation(kw[:], lg2[:], AF.Exp)
                    kw_r = rows.tile([1, SC], f32r, tag="kw_r")
                    nc.vector.tensor_copy(kw_r[:], kw[:])
                    kb_ps = psB.tile([128, SC], f32, tag="bcast")
                    nc.tensor.matmul(kb_ps[:], ones_k1[:], kw_r[:],
                                     start=True, stop=True)
                    kw_b = bc.tile([128, SC], f32, tag="kw_b")
                    nc.scalar.copy(kw_b[:], kb_ps[:])

                    den2 = rows.tile([1, SC], f32, tag="den2")
                    init2 = 0.0 if c == 0 else carry_d[:, 1:2]
                    nc.vector.tensor_tensor_scan(
                        den2[:], kw[:], kw[:], init2, OP.add, OP.bypass)
                    nc.vector.tensor_copy(carry_d[:, 1:2], den2[:, SC - 1:SC])
                    rden2 = rows.tile([1, SC], f32, tag="rden2")
                    rscr2 = rows.tile([1, SC], f32, tag="rscr2")
                    nc.vector.reciprocal_approx_accurate(rden2[:], den2[:], rscr2[:])
                    rden2_r = rows.tile([1, SC], f32r, tag="rden2_r")
                    nc.vector.tensor_copy(rden2_r[:], rden2[:])
                    d2_ps = psB.tile([128, SC], f32, tag="bcast")
                    nc.tensor.matmul(d2_ps[:], ones_k1[:], rden2_r[:],
                                     start=True, stop=True)
                    rden2_b = bc.tile([128, SC], f32, tag="rden2_b")
                    nc.scalar.copy(rden2_b[:], d2_ps[:])

                    for e in range(NE):
                        mki_t = wk2.tile([128, SC], bf16, tag="mki")
                        nc.sync.dma_start(
                            out=mki_t[:],
                            in_=mk_dram[e * 128:(e + 1) * 128, s0:s0 + SC])
                        u2_t = wk2.tile([128, SC], f32, tag="u2")
                        nc.vector.tensor_mul(u2_t[:], kw_b[:], mki_t[:])
                        n2_t = wk2.tile([128, SC], f32, tag="n2")
                        initk = 0.0 if c == 0 else carry_k[:, e:e + 1]
                        nc.vector.tensor_tensor_scan(
                            n2_t[:], u2_t[:], u2_t[:], initk, OP.add, OP.bypass)
                        nc.vector.tensor_copy(carry_k[:, e:e + 1],
                                              n2_t[:, SC - 1:SC])
                        pk_t = wk2.tile([128, SC], f32, tag="pk")
                        nc.gpsimd.tensor_mul(pk_t[:], n2_t[:], rden2_b[:])
                        qi_t = wk2.tile([128, SC], bf16, tag="qi")
                        nc.sync.dma_start(
                            out=qi_t[:],
                            in_=q_dram[e * 128:(e + 1) * 128, s0:s0 + SC])
                        o_t = wk2.tile([128, SC], f32, tag="o")
                        nc.vector.tensor_mul(o_t[:], pk_t[:], qi_t[:])
                        nc.sync.dma_start(
                            out=outT[e * 128:(e + 1) * 128, s0:s0 + SC], in_=o_t[:])

    nc.finalize()
    _prog_cache["nc"] = nc
    return nc


def _host_prep(hidden_states, attention_mask, Wq, wq_att, Wk, wk_att, ln_g, ln_b):
    """Build the 8 per-core input maps."""
    f4 = np.float32
    g = np.asarray(ln_g, f4)
    bb = np.asarray(ln_b, f4)
    Wq = np.asarray(Wq, f4)
    Wk = np.asarray(Wk, f4)
    wq_att = np.asarray(wq_att, f4)[:, 0]
    wk_att = np.asarray(wk_att, f4)[:, 0]
    h = np.asarray(hidden_states, f4)
    am = np.asarray(attention_mask, f4)

    Wqp = Wq * g[None, :]           # [e,d]
    Wkp = Wk * g[None, :]
    wqT_full = np.ascontiguousarray(Wqp.T)   # [d,e]
    wkT_full = np.ascontiguousarray(Wkp.T)
    cq_full = Wq @ bb               # [e]
    ck_full = Wk @ bb
    colsq_full = Wqp.sum(axis=1)    # [e]
    colsk_full = Wkp.sum(axis=1)

    vq = Wq.T @ wq_att              # [d]
    vqp = (g * vq) * INV_SQRT_D     # [d]
    cvq = float(bb @ vq) * INV_SQRT_D
    colsvq = np.array([[vqp.sum()]], f4)
    wkp_full = wk_att * INV_SQRT_D  # [e]

    maskb = (1.0 - am) * -10000.0   # [B,S]

    in_maps = []
    for core in range(NC):
        b, half = divmod(core, 2)
        sl = slice(half * EH, (half + 1) * EH)
        in_maps.append({
            "hT": np.ascontiguousarray(h[b].T),
            "wqT": np.ascontiguousarray(wqT_full[:, sl]),
            "wkT": np.ascontiguousarray(wkT_full[:, sl]),
            "cq": np.ascontiguousarray(cq_full[sl].reshape(NE, 128)),
            "ck": np.ascontiguousarray(ck_full[sl].reshape(NE, 128)),
            "vqp": np.ascontiguousarray(vqp.reshape(ND, 128)),
            "wkp": np.ascontiguousarray(wkp_full[sl].reshape(NE, 128)),
            "colsq": np.ascontiguousarray(colsq_full[sl].reshape(1, EH)),
            "colsk": np.ascontiguousarray(colsk_full[sl].reshape(1, EH)),
            "colsvq": colsvq,
            "mrow1": np.ascontiguousarray((maskb[b] + cvq).reshape(1, S)),
            "mrow2": np.ascontiguousarray(maskb[b].reshape(1, S)),
            "ones": np.ones((1, 128), f4),
        })
    return in_maps


def kernel(**inputs):
    nc = _build_program()
    in_maps = _host_prep(**inputs)
    res = run_bass_kernel_spmd(nc, in_maps, core_ids=list(range(NC)))
    out = np.empty((B, S, D), np.float32)
    for core in range(NC):
        b, half = divmod(core, 2)
        out[b, :, half * EH:(half + 1) * EH] = res.results[core]["outT"].T
    return out


# revision 14
# speedup vs baseline: 1.0193x; 1.0193x over previous
"""FastSelfAttention Trainium2 kernel.

Reference computation (B=4, S=4096, D=1024):
    h  = layer_norm(hidden_states, g, b)
    q  = h @ Wq.T ; k = h @ Wk.T ; v = q
    qw = exp((q @ wq_att) / sqrt(D) + mask)
    pq = cumsum(qw * q, S) / cumsum(qw, S)
    mk = pq * k
    kw = exp((mk @ wk_att) / sqrt(D) + mask)
    pk = cumsum(kw * mk, S) / cumsum(kw, S)
    out = pk * v

Sharding: 8 cores = 4 batches x 2 halves of the feature (e) dimension.
Each core owns its batch's full sequence and half of the q/k output
features. Layout on device is feature-major [e, s]; cumsum runs along
the free (s) axis via the DVE tensor_tensor_scan primitive, chained
across s-chunks with carry columns.

LayerNorm folding: with xs[d,s] = h[d,s]*rstd[s] (pre-scaled moving
operand) the projection is
    q[e,s] = sum_d W'q[e,d] xs[d,s] + (-mu[s]*rstd[s]) colsq[e] + cq[e]
so the -mu and +cq terms are rank-1 matmuls accumulated into the same
PSUM tile and the eviction is a plain copy (down to bf16).

The second pooling's logit l2[s] = sum_e wk[e]*mk[e,s] needs the full
e range: each core computes its half and a pairwise AllReduce
([[0,1],[2,3],[4,5],[6,7]]) combines them.
"""

import numpy as np
import ml_dtypes

import concourse.bass as bass
import concourse.bacc as bacc
import concourse.mybir as mybir
import concourse.tile as tile
from concourse.bass_utils import run_bass_kernel_spmd

dt = mybir.dt
AF = mybir.ActivationFunctionType
OP = mybir.AluOpType

B, S, D = 4, 4096, 1024
EH = D // 2          # e-half per core
NC = 8               # cores
SC = 512             # s-chunk
NSC = S // SC        # 8 s-chunks
ND = D // 128        # 8 d-chunks
NE = EH // 128       # 4 e-chunks per core
INV_SQRT_D = 1.0 / np.sqrt(np.float32(D))
EPS = 1e-5

_prog_cache = {}


def _build_program(nsc=NSC):
    key = ("nc", nsc)
    if key in _prog_cache:
        return _prog_cache[key]

    nc = bacc.Bacc("TRN2", num_devices=NC)
    f32, f32r, bf16 = dt.float32, dt.float32r, dt.bfloat16

    # ---- external I/O ----
    hT = nc.dram_tensor("hT", [D, S], f32, kind="ExternalInput")
    wqT = nc.dram_tensor("wqT", [D, EH], bf16, kind="ExternalInput")
    wkT = nc.dram_tensor("wkT", [D, EH], bf16, kind="ExternalInput")
    # per-partition constants, host layout [n, 128] -> SBUF [128, n]
    vqp_in = nc.dram_tensor("vqp", [ND, 128], bf16, kind="ExternalInput")
    wkp_in = nc.dram_tensor("wkp", [NE, 128], bf16, kind="ExternalInput")
    # rows
    colsq_in = nc.dram_tensor("colsq", [1, EH], bf16, kind="ExternalInput")
    colsk_in = nc.dram_tensor("colsk", [1, EH], bf16, kind="ExternalInput")
    colsvq_in = nc.dram_tensor("colsvq", [1, 1], bf16, kind="ExternalInput")
    cqr_in = nc.dram_tensor("cqr", [1, EH], bf16, kind="ExternalInput")
    ckr_in = nc.dram_tensor("ckr", [1, EH], bf16, kind="ExternalInput")
    mrow1_in = nc.dram_tensor("mrow1", [1, S], f32, kind="ExternalInput")
    mrow2_in = nc.dram_tensor("mrow2", [1, S], f32, kind="ExternalInput")
    ones_in = nc.dram_tensor("ones", [1, SC], bf16, kind="ExternalInput")
    ones32_in = nc.dram_tensor("ones32", [1, 128], f32, kind="ExternalInput")

    outT = nc.dram_tensor("outT", [EH, S], f32, kind="ExternalOutput")

    with tile.TileContext(nc) as tc:
        with (
            tc.tile_pool(name="const", bufs=1) as cpool,
            tc.tile_pool(name="persist", bufs=1) as ppool,
            tc.tile_pool(name="rows", bufs=2) as rows,
            tc.tile_pool(name="bc", bufs=2) as bc,
            tc.tile_pool(name="psA", bufs=2, space="PSUM") as psA,
            tc.tile_pool(name="psB", bufs=2, space="PSUM") as psB,
            tc.tile_pool(name="psR", bufs=2, space="PSUM") as psR,
            tc.tile_pool(name="psL2", bufs=1, space="PSUM") as psL2,
            tc.tile_pool(name="dram", bufs=1, space="DRAM") as dpool,
        ):
            # ---- constants (resident) ----
            vqp_t = cpool.tile([128, ND], bf16, tag="vqp")
            wkp_t = cpool.tile([128, NE], bf16, tag="wkp")
            nc.gpsimd.dma_start(out=vqp_t[:], in_=vqp_in.transpose([1, 0]))
            nc.gpsimd.dma_start(out=wkp_t[:], in_=wkp_in.transpose([1, 0]))

            colsq_t = cpool.tile([1, EH], bf16, tag="colsq")
            colsk_t = cpool.tile([1, EH], bf16, tag="colsk")
            colsvq_t = cpool.tile([1, 1], bf16, tag="colsvq")
            cqr_t = cpool.tile([1, EH], bf16, tag="cqr")
            ckr_t = cpool.tile([1, EH], bf16, tag="ckr")
            nc.gpsimd.dma_start(out=colsq_t[:], in_=colsq_in[:])
            nc.gpsimd.dma_start(out=colsk_t[:], in_=colsk_in[:])
            nc.gpsimd.dma_start(out=colsvq_t[:], in_=colsvq_in[:])
            nc.gpsimd.dma_start(out=cqr_t[:], in_=cqr_in[:])
            nc.gpsimd.dma_start(out=ckr_t[:], in_=ckr_in[:])

            ones_row = cpool.tile([1, SC], bf16, tag="ones_row")
            nc.gpsimd.dma_start(out=ones_row[:], in_=ones_in[:])
            ones_d = cpool.tile([128, 1], f32r, tag="ones_d")
            nc.gpsimd.dma_start(
                out=ones_d[:], in_=ones32_in.transpose([1, 0]).bitcast(f32r))
            ones_rk1 = cpool.tile([1, 128], f32r, tag="ones_rk1")
            nc.gpsimd.dma_start(out=ones_rk1[:], in_=ones32_in[:].bitcast(f32r))
            eps_t = cpool.tile([1, 1], f32, tag="eps")
            nc.vector.memset(eps_t[:], EPS)

            # ---- persistent state ----
            carry_q = ppool.tile([128, NE], f32, tag="carry_q")
            carry_k = ppool.tile([128, NE], f32, tag="carry_k")
            carry_d = ppool.tile([1, 2], f32, tag="carry_d")
            nc.vector.memset(carry_q[:], 0.0)
            nc.vector.memset(carry_k[:], 0.0)
            nc.vector.memset(carry_d[:], 0.0)

            l2p_dram = dpool.tile([1, S], f32, tag="l2p")
            l2f_dram = dpool.tile([1, S], f32, tag="l2f")
            q_dram = dpool.tile([EH, S], bf16, tag="q_dram")
            mk_dram = dpool.tile([EH, S], bf16, tag="mk_dram")

            # ================= sweep 1 =================
            with (
                tc.tile_pool(name="wpool", bufs=1) as wpool,
                tc.tile_pool(name="ht", bufs=2) as htpool,
                tc.tile_pool(name="wk1", bufs=2) as wk1,
            ):
                wq_t = wpool.tile([128, ND, EH], bf16, tag="wq")
                wk_t = wpool.tile([128, ND, EH], bf16, tag="wk")
                for d in range(ND):
                    nc.gpsimd.dma_start(
                        out=wq_t[:, d, :], in_=wqT[d * 128:(d + 1) * 128, :])
                    nc.gpsimd.dma_start(
                        out=wk_t[:, d, :], in_=wkT[d * 128:(d + 1) * 128, :])

                for c in range(nsc):
                    s0 = c * SC
                    ht_t = htpool.tile([128, ND, SC], f32r, tag="ht")
                    for d in range(ND):
                        nc.sync.dma_start(
                            out=ht_t[:, d, :],
                            in_=hT[d * 128:(d + 1) * 128, s0:s0 + SC].bitcast(f32r))

                    # ---- stats ----
                    sx_ps = psR.tile([1, SC], f32, tag="srow")
                    for d in range(ND):
                        nc.tensor.matmul(sx_ps[:], ones_d[:], ht_t[:, d, :],
                                         start=(d == 0), stop=(d == ND - 1))
                    sxx_ps = psR.tile([1, SC], f32, tag="srow")
                    for d in range(ND):
                        sq_t = wk1.tile([128, SC], f32r, tag="sq")
                        if d % 2 == 0:
                            nc.scalar.activation(
                                sq_t[:], ht_t[:, d, :].bitcast(f32), AF.Square)
                        else:
                            nc.vector.tensor_mul(
                                sq_t[:], ht_t[:, d, :].bitcast(f32),
                                ht_t[:, d, :].bitcast(f32))
                        nc.tensor.matmul(sxx_ps[:], ones_d[:], sq_t[:],
                                         start=(d == 0), stop=(d == ND - 1))

                    negmu = rows.tile([1, SC], f32, tag="negmu")
                    nc.vector.tensor_scalar_mul(negmu[:], sx_ps[:], -1.0 / D)
                    musq = rows.tile([1, SC], f32, tag="musq")
                    nc.scalar.activation(musq[:], sx_ps[:], AF.Square, scale=1.0 / D)
                    var = rows.tile([1, SC], f32, tag="var")
                    nc.vector.scalar_tensor_tensor(
                        var[:], sxx_ps[:], 1.0 / D, musq[:], OP.mult, OP.subtract)
                    sd = rows.tile([1, SC], f32, tag="sd")
                    nc.scalar.activation(sd[:], var[:], AF.Sqrt, bias=eps_t[:])
                    rstd = rows.tile([1, SC], f32, tag="rstd")
                    rscr = rows.tile([1, SC], f32, tag="rscr")
                    nc.vector.reciprocal_approx_accurate(rstd[:], sd[:], rscr[:])
                    rstd_r = rows.tile([1, SC], bf16, tag="rstd_r")
                    nc.vector.tensor_copy(rstd_r[:], rstd[:])
                    # -mu*rstd row for the rank-1 LN correction
                    nmur = rows.tile([1, SC], bf16, tag="nmur")
                    nc.vector.tensor_mul(nmur[:], negmu[:], rstd[:])

                    rb_ps = psB.tile([128, SC], f32, tag="bcast")
                    nc.tensor.matmul(rb_ps[:], ones_row[:, 0:128], rstd_r[:],
                                     start=True, stop=True)
                    rstd_b = bc.tile([128, SC], f32, tag="rstd_b")
                    nc.scalar.copy(rstd_b[:], rb_ps[:])

                    # pre-scaled moving operand: xs = ht * rstd (bf16)
                    xs_t = wk1.tile([128, ND, SC], bf16, tag="xs")
                    for d in range(ND):
                        if d < 5:
                            nc.vector.tensor_mul(
                                xs_t[:, d, :], ht_t[:, d, :].bitcast(f32), rstd_b[:])
                        else:
                            nc.gpsimd.tensor_mul(
                                xs_t[:, d, :], ht_t[:, d, :].bitcast(f32), rstd_b[:])

                    # ---- l1 row (query attention logit) ----
                    l1_ps = psR.tile([1, SC], f32, tag="srow")
                    for d in range(ND):
                        nc.tensor.matmul(l1_ps[:], vqp_t[:, d:d + 1], xs_t[:, d, :],
                                         start=(d == 0), stop=False)
                    nc.tensor.matmul(l1_ps[:], colsvq_t[:], nmur[:],
                                     start=False, stop=True)
                    l1b = rows.tile([1, SC], f32, tag="l1b")
                    m1s = rows.tile([1, SC], f32, tag="m1s")
                    nc.sync.dma_start(out=m1s[:], in_=mrow1_in[:, s0:s0 + SC])
                    nc.vector.tensor_add(l1b[:], l1_ps[:], m1s[:])
                    qw = rows.tile([1, SC], bf16, tag="qw")
                    nc.scalar.activation(qw[:], l1b[:], AF.Exp)

                    qb_ps = psB.tile([128, SC], f32, tag="bcast")
                    nc.tensor.matmul(qb_ps[:], ones_row[:, 0:128], qw[:],
                                     start=True, stop=True)
                    qw_b = bc.tile([128, SC], bf16, tag="qw_b")
                    nc.scalar.copy(qw_b[:], qb_ps[:])

                    # den1 scan + reciprocal + broadcast
                    den1 = rows.tile([1, SC], f32, tag="den1")
                    init1 = 0.0 if c == 0 else carry_d[:, 0:1]
                    nc.vector.tensor_tensor_scan(
                        den1[:], qw[:], qw[:], init1, OP.add, OP.bypass)
                    nc.vector.tensor_copy(carry_d[:, 0:1], den1[:, SC - 1:SC])
                    rden1 = rows.tile([1, SC], f32, tag="rden1")
                    nc.vector.reciprocal_approx_accurate(rden1[:], den1[:], rscr[:])
                    rden1h = rows.tile([1, SC], bf16, tag="rden1h")
                    nc.vector.tensor_copy(rden1h[:], rden1[:])
                    db_ps = psB.tile([128, SC], f32, tag="bcast")
                    nc.tensor.matmul(db_ps[:], ones_row[:, 0:128], rden1h[:],
                                     start=True, stop=True)
                    rden1_b = bc.tile([128, SC], f32, tag="rden1_b")
                    nc.scalar.copy(rden1_b[:], db_ps[:])

                    # ---- per e-chunk: projections, pool1, mk, l2 partial ----
                    l2_ps = psL2.tile([1, SC], f32, tag="l2")
                    for e in range(NE):
                        es = slice(e * 128, (e + 1) * 128)
                        qmm_ps = psA.tile([128, SC], f32, tag="proj")
                        for d in range(ND):
                            nc.tensor.matmul(
                                qmm_ps[:], wq_t[:, d, es], xs_t[:, d, :],
                                start=(d == 0), stop=False)
                        nc.tensor.matmul(qmm_ps[:], colsq_t[:, es], nmur[:],
                                         start=False, stop=False)
                        nc.tensor.matmul(qmm_ps[:], cqr_t[:, es], ones_row[:],
                                         start=False, stop=True)
                        q_t = wk1.tile([128, SC], bf16, tag="q")
                        nc.scalar.copy(q_t[:], qmm_ps[:])
                        nc.sync.dma_start(
                            out=q_dram[es, s0:s0 + SC], in_=q_t[:])

                        kmm_ps = psA.tile([128, SC], f32, tag="proj")
                        for d in range(ND):
                            nc.tensor.matmul(
                                kmm_ps[:], wk_t[:, d, es], xs_t[:, d, :],
                                start=(d == 0), stop=False)
                        nc.tensor.matmul(kmm_ps[:], colsk_t[:, es], nmur[:],
                                         start=False, stop=False)
                        nc.tensor.matmul(kmm_ps[:], ckr_t[:, es], ones_row[:],
                                         start=False, stop=True)
                        k_t = wk1.tile([128, SC], bf16, tag="k")
                        nc.scalar.copy(k_t[:], kmm_ps[:])

                        u1_t = wk1.tile([128, SC], bf16, tag="u1")
                        nc.vector.tensor_mul(u1_t[:], qw_b[:], q_t[:])
                        n1_t = wk1.tile([128, SC], f32, tag="n1")
                        initq = 0.0 if c == 0 else carry_q[:, e:e + 1]
                        nc.vector.tensor_tensor_scan(
                            n1_t[:], u1_t[:], u1_t[:], initq, OP.add, OP.bypass)
                        nc.vector.tensor_copy(carry_q[:, e:e + 1], n1_t[:, SC - 1:SC])

                        pq_t = wk1.tile([128, SC], bf16, tag="pq")
                        nc.gpsimd.tensor_mul(pq_t[:], n1_t[:], rden1_b[:])
                        mk_t = wk1.tile([128, SC], bf16, tag="mk")
                        nc.gpsimd.tensor_mul(mk_t[:], pq_t[:], k_t[:])
                        nc.sync.dma_start(
                            out=mk_dram[es, s0:s0 + SC], in_=mk_t[:])
                        nc.tensor.matmul(l2_ps[:], wkp_t[:, e:e + 1], mk_t[:],
                                         start=(e == 0), stop=(e == NE - 1))

                    l2p_row = rows.tile([1, SC], f32, tag="l2p")
                    nc.vector.tensor_copy(l2p_row[:], l2_ps[:])
                    nc.sync.dma_start(out=l2p_dram[:, s0:s0 + SC], in_=l2p_row[:])

            # ================= allreduce =================
            nc.gpsimd.collective_compute(
                "AllReduce", OP.add,
                replica_groups=[[0, 1], [2, 3], [4, 5], [6, 7]],
                ins=[l2p_dram[:]], outs=[l2f_dram[:]],
            )

            # ================= sweep 2 =================
            with tc.tile_pool(name="wk2", bufs=2) as wk2:
                for c in range(nsc):
                    s0 = c * SC
                    l2s = rows.tile([1, SC], f32, tag="l2s")
                    nc.sync.dma_start(out=l2s[:], in_=l2f_dram[:, s0:s0 + SC])
                    m2s = rows.tile([1, SC], f32, tag="m2s")
                    nc.sync.dma_start(out=m2s[:], in_=mrow2_in[:, s0:s0 + SC])
                    lg2 = rows.tile([1, SC], f32, tag="lg2")
                    nc.vector.tensor_add(lg2[:], l2s[:], m2s[:])
                    kw = rows.tile([1, SC], bf16, tag="kw")
                    nc.scalar.activation(kw[:], lg2[:], AF.Exp)
                    kb_ps = psB.tile([128, SC], f32, tag="bcast")
                    nc.tensor.matmul(kb_ps[:], ones_row[:, 0:128], kw[:],
                                     start=True, stop=True)
                    kw_b = bc.tile([128, SC], bf16, tag="kw_b")
                    nc.scalar.copy(kw_b[:], kb_ps[:])

                    den2 = rows.tile([1, SC], f32, tag="den2")
                    init2 = 0.0 if c == 0 else carry_d[:, 1:2]
                    nc.vector.tensor_tensor_scan(
                        den2[:], kw[:], kw[:], init2, OP.add, OP.bypass)
                    nc.vector.tensor_copy(carry_d[:, 1:2], den2[:, SC - 1:SC])
                    rden2 = rows.tile([1, SC], f32, tag="rden2")
                    rscr2 = rows.tile([1, SC], f32, tag="rscr2")
                    nc.vector.reciprocal_approx_accurate(rden2[:], den2[:], rscr2[:])
                    rden2h = rows.tile([1, SC], bf16, tag="rden2h")
                    nc.vector.tensor_copy(rden2h[:], rden2[:])
                    d2_ps = psB.tile([128, SC], f32, tag="bcast")
                    nc.tensor.matmul(d2_ps[:], ones_row[:, 0:128], rden2h[:],
                                     start=True, stop=True)
                    rden2_b = bc.tile([128, SC], f32, tag="rden2_b")
                    nc.scalar.copy(rden2_b[:], d2_ps[:])

                    for e in range(NE):
                        es = slice(e * 128, (e + 1) * 128)
                        mki_t = wk2.tile([128, SC], bf16, tag="mki")
                        nc.sync.dma_start(out=mki_t[:],
                                          in_=mk_dram[es, s0:s0 + SC])
                        u2_t = wk2.tile([128, SC], bf16, tag="u2")
                        nc.vector.tensor_mul(u2_t[:], kw_b[:], mki_t[:])
                        n2_t = wk2.tile([128, SC], f32, tag="n2")
                        initk = 0.0 if c == 0 else carry_k[:, e:e + 1]
                        nc.vector.tensor_tensor_scan(
                            n2_t[:], u2_t[:], u2_t[:], initk, OP.add, OP.bypass)
                        nc.vector.tensor_copy(carry_k[:, e:e + 1],
                                              n2_t[:, SC - 1:SC])
                        pk_t = wk2.tile([128, SC], f32, tag="pk")
                        nc.gpsimd.tensor_mul(pk_t[:], n2_t[:], rden2_b[:])
                        qi_t = wk2.tile([128, SC], bf16, tag="qi")
                        nc.sync.dma_start(out=qi_t[:],
                                          in_=q_dram[es, s0:s0 + SC])
                        o_t = wk2.tile([128, SC], f32, tag="o")
                        nc.vector.tensor_mul(o_t[:], pk_t[:], qi_t[:])
                        nc.sync.dma_start(
                            out=outT[es, s0:s0 + SC], in_=o_t[:])

    nc.finalize()
    _prog_cache[key] = nc
    return nc


def _host_prep(hidden_states, attention_mask, Wq, wq_att, Wk, wk_att, ln_g, ln_b):
    """Build the 8 per-core input maps."""
    f4 = np.float32
    g = np.asarray(ln_g, f4)
    bb = np.asarray(ln_b, f4)
    Wq = np.asarray(Wq, f4)
    Wk = np.asarray(Wk, f4)
    wq_att = np.asarray(wq_att, f4)[:, 0]
    wk_att = np.asarray(wk_att, f4)[:, 0]
    h = np.asarray(hidden_states, f4)
    am = np.asarray(attention_mask, f4)

    Wqp = Wq * g[None, :]           # [e,d]
    Wkp = Wk * g[None, :]
    wqT_full = np.ascontiguousarray(Wqp.T)   # [d,e]
    wkT_full = np.ascontiguousarray(Wkp.T)
    cq_full = Wq @ bb               # [e]
    ck_full = Wk @ bb
    colsq_full = Wqp.sum(axis=1)    # [e]
    colsk_full = Wkp.sum(axis=1)

    vq = Wq.T @ wq_att              # [d]
    vqp = (g * vq) * INV_SQRT_D     # [d]
    cvq = float(bb @ vq) * INV_SQRT_D
    colsvq = np.array([[vqp.sum()]], f4)
    wkp_full = (wk_att * INV_SQRT_D).astype(f4)

    maskb = (1.0 - am) * -10000.0   # [B,S]

    def bf(a):
        return np.ascontiguousarray(np.asarray(a, f4).astype(ml_dtypes.bfloat16))

    in_maps = []
    for core in range(NC):
        b, half = divmod(core, 2)
        sl = slice(half * EH, (half + 1) * EH)
        in_maps.append({
            "hT": np.ascontiguousarray(h[b].T),
            "wqT": bf(wqT_full[:, sl]),
            "wkT": bf(wkT_full[:, sl]),
            "vqp": bf(vqp.reshape(ND, 128)),
            "wkp": bf(wkp_full[sl].reshape(NE, 128)),
            "colsq": bf(colsq_full[sl].reshape(1, EH)),
            "colsk": bf(colsk_full[sl].reshape(1, EH)),
            "colsvq": bf(colsvq),
            "cqr": bf(cq_full[sl].reshape(1, EH)),
            "ckr": bf(ck_full[sl].reshape(1, EH)),
            "mrow1": np.ascontiguousarray((maskb[b] + cvq).reshape(1, S)),
            "mrow2": np.ascontiguousarray(maskb[b].reshape(1, S)),
            "ones": bf(np.ones((1, SC), f4)),
            "ones32": np.ones((1, 128), f4),
        })
    return in_maps


def kernel(**inputs):
    nc = _build_program()
    in_maps = _host_prep(**inputs)
    res = run_bass_kernel_spmd(nc, in_maps, core_ids=list(range(NC)))
    out = np.empty((B, S, D), np.float32)
    for core in range(NC):
        b, half = divmod(core, 2)
        out[b, :, half * EH:(half + 1) * EH] = res.results[core]["outT"].T
    return out
